# revision 3
# baseline (speedup 1.0000x reference)
"""2-layer GraphSAGE (mean aggr) on 8 Trainium2 NeuronCores.

Strategy: partition destination nodes across cores (graph parallel).
Segment-mean is computed as TensorE matmuls: for each tile of 128 gathered
source rows M [128e, 128k], a routing matrix S [128e, 128d] (one-hot by
local destination, scaled by 1/deg) accumulates aggT[k,d] += M.T @ S into
PSUM per 128-node destination block. Source rows are fetched with
dma_gather (int16 indices -> 4 source chunks of 25000 rows). Linear layers
and bias are fused per block on TensorE. Layer 1 and layer 2 run as two
launches; the host assembles the full hidden table in between (the
inter-core exchange).
"""

import sys

sys.path.insert(0, "/opt/trn_rl_repo")

import numpy as np

import concourse.mybir as mybir
import concourse.tile as tile
from concourse import bacc, bass_utils

N_NODES = 100000
N_EDGES = 1600000
IN_DIM = 128
HID_DIM = 128
OUT_DIM = 64
N_CORES = 8
BLOCK = 128
N_CHUNKS = 4
CHUNK_SZ = 25000
GATHER_MAX = 1024  # HW limit: dma_gather wedges above this

_plan_cache: dict = {}
_prog_cache: dict = {}


def _make_plan(edge_index, n_nodes, n_cores, chunk_sz, n_chunks):
    src = np.asarray(edge_index[0], dtype=np.int64)
    dst = np.asarray(edge_index[1], dtype=np.int64)
    n_edges = src.shape[0]

    deg = np.bincount(dst, minlength=n_nodes).astype(np.int64)
    cnt_inv = (1.0 / np.maximum(deg, 1)).astype(np.float32)

    # Balanced blocks: degree-sorted snake round-robin over all blocks.
    n_blocks_total = -(-n_nodes // BLOCK)
    while n_blocks_total % n_cores:
        n_blocks_total += 1
    bpc = n_blocks_total // n_cores
    slots_per_core = bpc * BLOCK
    order = np.argsort(-deg, kind="stable")
    i = np.arange(n_nodes)
    r = i // n_blocks_total
    b = i % n_blocks_total
    b = np.where(r % 2 == 0, b, n_blocks_total - 1 - b)
    slot = b * BLOCK + r
    slot_of_node = np.empty(n_nodes, np.int64)
    slot_of_node[order] = slot

    dslot = slot_of_node[dst]
    core_e = dslot // slots_per_core
    blk_e = (dslot % slots_per_core) // BLOCK
    dloc_e = dslot % BLOCK
    chunk_e = src // chunk_sz

    cell = (core_e * bpc + blk_e) * n_chunks + chunk_e
    n_cells = n_cores * bpc * n_chunks
    counts = np.bincount(cell, minlength=n_cells).reshape(
        n_cores, bpc, n_chunks)
    T = -(-counts.max(axis=0) // BLOCK)          # [bpc, n_chunks] tiles/cell

    cell_slots = (T * BLOCK).astype(np.int64)
    seg_len = cell_slots.sum(axis=0)             # per chunk
    seg_start = np.concatenate([[0], np.cumsum(seg_len)[:-1]])
    cell_base = np.empty((bpc, n_chunks), np.int64)
    for c in range(n_chunks):
        cell_base[:, c] = seg_start[c] + np.concatenate(
            [[0], np.cumsum(cell_slots[:, c])[:-1]])
    total_slots = int(cell_slots.sum())

    gathers = []
    for c in range(n_chunks):
        lst = []
        off = 0
        while off < seg_len[c]:
            n = int(min(GATHER_MAX, seg_len[c] - off))
            lst.append((int(seg_start[c] + off), n))
            off += n
        gathers.append(lst)

    # slot position of every edge
    eorder = np.argsort(cell, kind="stable")
    sorted_cell = cell[eorder]
    group_start = np.zeros(n_edges, np.int64)
    new_grp = np.empty(n_edges, bool)
    new_grp[0] = True
    new_grp[1:] = sorted_cell[1:] != sorted_cell[:-1]
    grp_first = np.where(new_grp)[0]
    group_start[grp_first] = grp_first
    group_start = np.maximum.accumulate(group_start)
    rank = np.arange(n_edges) - group_start

    b_of = (sorted_cell // n_chunks) % bpc
    c_of = sorted_cell % n_chunks
    core_of = sorted_cell // (bpc * n_chunks)
    pos = cell_base[b_of, c_of] + rank

    idx_vals = np.zeros((n_cores, total_slots), np.int16)
    dloc_vals = np.full((n_cores, total_slots), -1.0, np.float32)
    cinv_vals = np.zeros((n_cores, total_slots), np.float32)

    es, ed = src[eorder], dst[eorder]
    idx_vals[core_of, pos] = (es - c_of * chunk_sz).astype(np.int16)
    dloc_vals[core_of, pos] = dloc_e[eorder].astype(np.float32)
    cinv_vals[core_of, pos] = cnt_inv[ed]

    idx16 = np.ascontiguousarray(
        np.tile(idx_vals.reshape(n_cores, -1, 16).transpose(0, 2, 1),
                (1, 8, 1)))
    dstloc = np.ascontiguousarray(
        dloc_vals.reshape(n_cores, -1, BLOCK).transpose(0, 2, 1))
    cntinv = np.ascontiguousarray(
        cinv_vals.reshape(n_cores, -1, BLOCK).transpose(0, 2, 1))

    return dict(
        slot_of_node=slot_of_node, bpc=bpc, slots_per_core=slots_per_core,
        T=T, gathers=gathers, total_slots=total_slots,
        cell_base=cell_base, seg_start=seg_start,
        idx16=idx16, dstloc=dstloc, cntinv=cntinv, chunk_sz=chunk_sz,
        n_chunks=n_chunks, n_nodes=n_nodes, n_cores=n_cores,
    )


def _build_program(plan, table_rows, out_d, relu):
    """One layer's SPMD program (shared by all cores)."""
    bpc = plan["bpc"]
    T = plan["T"]
    n_chunks = plan["n_chunks"]
    chunk_sz = plan["chunk_sz"]
    total_slots = plan["total_slots"]
    slots_pc = plan["slots_per_core"]
    gathers = plan["gathers"]
    cell_base = plan["cell_base"]
    seg_start = plan["seg_start"]
    D = 128

    nc = bacc.Bacc("TRN2", target_bir_lowering=False, debug=False)
    with tile.TileContext(nc) as tc:
        with tc.tile_pool(name="dram", bufs=1, space="DRAM") as dram:
            table = dram.tile([table_rows, D], mybir.dt.float32,
                              kind="ExternalInput", name="table")
            idx16 = dram.tile([128, total_slots // 16], mybir.dt.int16,
                              kind="ExternalInput", name="idx16")
            dstloc = dram.tile([128, total_slots // BLOCK], mybir.dt.float32,
                               kind="ExternalInput", name="dstloc")
            cntinv = dram.tile([128, total_slots // BLOCK], mybir.dt.float32,
                               kind="ExternalInput", name="cntinv")
            xT = dram.tile([D, slots_pc], mybir.dt.float32,
                           kind="ExternalInput", name="xT")
            wl = dram.tile([D, out_d], mybir.dt.float32,
                           kind="ExternalInput", name="wl")
            wr = dram.tile([D, out_d], mybir.dt.float32,
                           kind="ExternalInput", name="wr")
            brow = dram.tile([1, out_d], mybir.dt.float32,
                             kind="ExternalInput", name="brow")
            iota_in = dram.tile([128, BLOCK], mybir.dt.float32,
                                kind="ExternalInput", name="iota")
            out = dram.tile([slots_pc, out_d], mybir.dt.float32,
                            kind="ExternalOutput", name="out")

        with tc.tile_pool(name="const", bufs=1) as cpool, \
             tc.tile_pool(name="gbuf", bufs=4) as gpool, \
             tc.tile_pool(name="spool", bufs=4) as spool, \
             tc.tile_pool(name="fpool", bufs=3) as fpool, \
             tc.tile_pool(name="psA", bufs=2, space="PSUM") as psA, \
             tc.tile_pool(name="psB", bufs=2, space="PSUM") as psB:

            idx_sb = cpool.tile([128, total_slots // 16], mybir.dt.int16)
            dst_sb = cpool.tile([128, total_slots // BLOCK], mybir.dt.float32)
            cnt_sb = cpool.tile([128, total_slots // BLOCK], mybir.dt.float32)
            xT_sb = cpool.tile([D, slots_pc], mybir.dt.float32)
            wl_sb = cpool.tile([D, out_d], mybir.dt.float32)
            wr_sb = cpool.tile([D, out_d], mybir.dt.float32)
            b_sb = cpool.tile([1, out_d], mybir.dt.float32)
            ones_sb = cpool.tile([1, BLOCK], mybir.dt.float32)
            iota_sb = cpool.tile([128, BLOCK], mybir.dt.float32)

            nc.sync.dma_start(out=idx_sb[:], in_=idx16[:])
            nc.sync.dma_start(out=dst_sb[:], in_=dstloc[:])
            nc.sync.dma_start(out=cnt_sb[:], in_=cntinv[:])
            nc.sync.dma_start(out=xT_sb[:], in_=xT[:])
            nc.sync.dma_start(out=wl_sb[:], in_=wl[:])
            nc.sync.dma_start(out=wr_sb[:], in_=wr[:])
            nc.sync.dma_start(out=b_sb[:], in_=brow[:])
            nc.sync.dma_start(out=iota_sb[:], in_=iota_in[:])
            nc.vector.memset(ones_sb[:], 1.0)

            gtiles = [dict() for _ in range(n_chunks)]
            next_g = [0] * n_chunks

            def ensure_gather(c, gi):
                while next_g[c] <= gi:
                    g = next_g[c]
                    s0, n = gathers[c][g]
                    gb = gpool.tile([128, GATHER_MAX // 128, D],
                                    mybir.dt.float32, tag=f"g{c}",
                                    name=f"gb_{c}_{g}")
                    nc.gpsimd.dma_gather(
                        out_ap=gb[:, : -(-n // 128), :],
                        in_ap=table[c * chunk_sz : min((c + 1) * chunk_sz,
                                                       table_rows), :],
                        idxs_ap=idx_sb[:, s0 // 16 : (s0 + n) // 16],
                        num_idxs=n,
                        num_idxs_reg=n,
                        elem_size=D,
                    )
                    gtiles[c][g] = gb
                    next_g[c] = g + 1

            for b in range(bpc):
                agg = psA.tile([D, BLOCK], mybir.dt.float32, space="PSUM",
                               tag="agg", name=f"agg_{b}")
                n_mm = int(T[b].sum())
                mm = 0
                for c in range(n_chunks):
                    tcount = int(T[b, c])
                    for t in range(tcount):
                        slot0 = int(cell_base[b, c]) + t * 128
                        g = (slot0 - int(seg_start[c])) // GATHER_MAX
                        tin = ((slot0 - int(seg_start[c])) % GATHER_MAX) // 128
                        ensure_gather(c, g)
                        gb = gtiles[c][g]
                        gt_col = slot0 // 128
                        s_tile = spool.tile([128, BLOCK], mybir.dt.float32,
                                            tag="s", name=f"s_{b}_{c}_{t}")
                        nc.vector.tensor_scalar(
                            out=s_tile[:],
                            in0=iota_sb[:],
                            scalar1=dst_sb[:, gt_col : gt_col + 1],
                            scalar2=cnt_sb[:, gt_col : gt_col + 1],
                            op0=mybir.AluOpType.is_equal,
                            op1=mybir.AluOpType.mult,
                        )
                        nc.tensor.matmul(
                            out=agg[:],
                            lhsT=gb[:, tin, :],
                            rhs=s_tile[:],
                            start=(mm == 0),
                            stop=(mm == n_mm - 1),
                        )
                        mm += 1

                outp = psB.tile([BLOCK, out_d], mybir.dt.float32,
                                space="PSUM", tag="outp", name=f"outp_{b}")
                if n_mm > 0:
                    aggc = fpool.tile([D, BLOCK], mybir.dt.float32,
                                      tag="aggc", name=f"aggc_{b}")
                    nc.scalar.copy(out=aggc[:], in_=agg[:])
                    nc.tensor.matmul(out=outp[:], lhsT=aggc[:], rhs=wl_sb[:],
                                     start=True, stop=False)
                    nc.tensor.matmul(
                        out=outp[:],
                        lhsT=xT_sb[:, b * BLOCK : (b + 1) * BLOCK],
                        rhs=wr_sb[:], start=False, stop=False)
                else:
                    nc.tensor.matmul(
                        out=outp[:],
                        lhsT=xT_sb[:, b * BLOCK : (b + 1) * BLOCK],
                        rhs=wr_sb[:], start=True, stop=False)
                nc.tensor.matmul(out=outp[:], lhsT=ones_sb[:], rhs=b_sb[:],
                                 start=False, stop=True)

                fin = fpool.tile([BLOCK, out_d], mybir.dt.float32,
                                 tag="fin", name=f"fin_{b}")
                if relu:
                    nc.vector.tensor_scalar(
                        out=fin[:], in0=outp[:], scalar1=0.0, scalar2=None,
                        op0=mybir.AluOpType.max)
                else:
                    nc.vector.tensor_copy(out=fin[:], in_=outp[:])
                nc.sync.dma_start(out=out[b * BLOCK : (b + 1) * BLOCK, :],
                                  in_=fin[:])

    nc.compile()
    names = dict(table=table.name, idx16=idx16.name, dstloc=dstloc.name,
                 cntinv=cntinv.name, xT=xT.name, wl=wl.name, wr=wr.name,
                 brow=brow.name, iota=iota_in.name, out=out.name)
    return nc, names


def _layer_in_maps(names, plan, table_np, xT_np, wlT, wrT, b_vec, out_d):
    iota = np.broadcast_to(
        np.arange(BLOCK, dtype=np.float32), (128, BLOCK)).copy()
    in_maps = []
    for c in range(plan["n_cores"]):
        in_maps.append({
            names["table"]: table_np,
            names["idx16"]: plan["idx16"][c],
            names["dstloc"]: plan["dstloc"][c],
            names["cntinv"]: plan["cntinv"][c],
            names["xT"]: xT_np[c],
            names["wl"]: wlT,
            names["wr"]: wrT,
            names["brow"]: np.ascontiguousarray(b_vec.reshape(1, out_d)),
            names["iota"]: iota,
        })
    return in_maps


def _run_layer(nc, names, plan, table_np, xT_np, wlT, wrT, b_vec, out_d):
    in_maps = _layer_in_maps(names, plan, table_np, xT_np, wlT, wrT, b_vec,
                             out_d)
    res = bass_utils.run_bass_kernel_spmd(
        nc, in_maps, core_ids=list(range(plan["n_cores"])))
    return [res.results[c][names["out"]] for c in range(plan["n_cores"])]


def _get_plan_and_progs(edge_index):
    key = hash(edge_index.tobytes())
    if key not in _plan_cache:
        _plan_cache[key] = _make_plan(edge_index, N_NODES, N_CORES,
                                      CHUNK_SZ, N_CHUNKS)
    plan = _plan_cache[key]
    if (key, "L1") not in _prog_cache:
        _prog_cache[(key, "L1")] = _build_program(plan, N_NODES, HID_DIM,
                                                  relu=True)
    if (key, "L2") not in _prog_cache:
        _prog_cache[(key, "L2")] = _build_program(plan, N_NODES, OUT_DIM,
                                                  relu=False)
    return plan, _prog_cache[(key, "L1")], _prog_cache[(key, "L2")]


def kernel(x, edge_index, W1l, b1, W1r, W2l, b2, W2r):
    x = np.asarray(x, np.float32)
    edge_index = np.asarray(edge_index)
    W1l = np.asarray(W1l, np.float32)
    b1 = np.asarray(b1, np.float32)
    W1r = np.asarray(W1r, np.float32)
    W2l = np.asarray(W2l, np.float32)
    b2 = np.asarray(b2, np.float32)
    W2r = np.asarray(W2r, np.float32)

    plan, (nc1, names1), (nc2, names2) = _get_plan_and_progs(edge_index)

    slot_of_node = plan["slot_of_node"]
    spc = plan["slots_per_core"]
    n_cores = plan["n_cores"]

    xq = np.zeros((n_cores * spc, IN_DIM), np.float32)
    xq[slot_of_node] = x
    xT_np = [np.ascontiguousarray(xq[c * spc : (c + 1) * spc].T)
             for c in range(n_cores)]

    h_parts = _run_layer(nc1, names1, plan, x, xT_np,
                         np.ascontiguousarray(W1l.T),
                         np.ascontiguousarray(W1r.T), b1, HID_DIM)

    hq = np.concatenate(h_parts, axis=0)
    h_full = np.ascontiguousarray(hq[slot_of_node])
    hT_np = [np.ascontiguousarray(h_parts[c].T) for c in range(n_cores)]

    out_parts = _run_layer(nc2, names2, plan, h_full, hT_np,
                           np.ascontiguousarray(W2l.T),
                           np.ascontiguousarray(W2r.T), b2, OUT_DIM)
    oq = np.concatenate(out_parts, axis=0)
    return np.ascontiguousarray(oq[slot_of_node]).astype(np.float32)
